# Initial kernel scaffold
#
"""CoarseMatching (retrieval kNN) kernel for 8x Trainium2 NeuronCores.

Problem: ref[8192,256], src[8192,256] (unit-norm rows, fp32).
  sim = ref @ src.T                      [8192, 8192]
  scores = exp(2*sim - 2)                (monotone in sim)
  outputs: global top-k (k=num_proposal) of scores (row idx, col idx, score)
           + per-row argmax over src.

Strategy:
  - Shard ref rows across 8 cores (1024 rows each); src replicated.
  - Device (per core): bf16 matmul (fp32 PSUM accumulation) of its
    [1024 x 8192] sim block; DVE max-reduce each PSUM group to per-row,
    per-512-column-chunk maxes "cm" [1024 x 16]. Only cm leaves the device.
  - Host: candidate selection from cm with a safety margin that dominates
    the bf16 rounding error, then exact fp64 recomputation of only the
    few hundred candidate chunks (BLAS) for exact top-k / argmax.

  Device cm error vs true fp32 sims is bounded by bf16 input rounding
  (~6e-4 absolute); MARGIN=2e-2 makes candidate selection exact.
"""

import sys

sys.path.insert(0, "/opt/trn_rl_repo")

import numpy as np
import ml_dtypes

N_CORES = 8
N, M, C = 8192, 8192, 256
ROWS_PER_CORE = N // N_CORES          # 1024
STRIPS = ROWS_PER_CORE // 128         # 8 strips of 128 rows
CHUNK = 512                           # column chunk = one PSUM bank of fp32
N_CHUNKS = M // CHUNK                 # 16
GROUP = 4                             # PSUM banks per reduce group
MARGIN = 2e-2                         # >> bf16 matmul error (~6e-4)

_compiled = None


def _build_bass():
    from contextlib import ExitStack
    import concourse.bass as bass
    import concourse.tile as tile
    from concourse import mybir

    nc = bass.Bass()
    bf16 = mybir.dt.bfloat16
    f32 = mybir.dt.float32

    # lhsT k-tiles: [2, 128, 1024] (contract dim on partitions)
    ref_t = nc.declare_dram_parameter("ref_t", [2, 128, ROWS_PER_CORE], bf16, isOutput=False)
    # rhs k-tiles quartered for load/compute overlap: [2, 4, 128, 2048]
    src_t = nc.declare_dram_parameter("src_t", [2, 4, 128, M // 4], bf16, isOutput=False)
    # out: per chunk j, [128 partitions, 8 strips] of chunk maxes
    cm_out = nc.declare_dram_parameter("cm", [N_CHUNKS, 128, STRIPS], f32, isOutput=True)

    with tile.TileContext(nc) as tc, ExitStack() as ctx:
        sbuf = ctx.enter_context(tc.tile_pool(name="sbuf", bufs=1))
        cm_pool = ctx.enter_context(tc.tile_pool(name="cmp", bufs=4))
        psum = ctx.enter_context(tc.tile_pool(name="psum", bufs=2, space="PSUM"))

        # resident weights (ref^T) per k-tile
        reft = [sbuf.tile([128, ROWS_PER_CORE], bf16, name=f"reft{k}") for k in range(2)]
        for k in range(2):
            nc.sync.dma_start(reft[k][:], ref_t[k])

        # resident src^T quarters per k-tile
        srcq = [
            [sbuf.tile([128, M // 4], bf16, name=f"srcq{k}_{q}") for q in range(4)]
            for k in range(2)
        ]
        for q in range(4):
            for k in range(2):
                nc.sync.dma_start(srcq[k][q][:], src_t[k, q])

        for j in range(N_CHUNKS):          # column chunks of 512
            q, off = j // 4, (j % 4) * CHUNK
            cm_sb = cm_pool.tile([128, STRIPS], f32, name="cm_sb", tag="cm_sb")
            for h in range(STRIPS // GROUP):   # two strip-halves
                ps = psum.tile([128, GROUP, CHUNK], f32, name="ps", tag="ps")
                for s in range(GROUP):
                    strip = h * GROUP + s
                    for k in range(2):
                        nc.tensor.matmul(
                            ps[:, s],
                            reft[k][:, strip * 128:(strip + 1) * 128],
                            srcq[k][q][:, off:off + CHUNK],
                            start=(k == 0),
                            stop=(k == 1),
                        )
                nc.vector.tensor_reduce(
                    cm_sb[:, h * GROUP:(h + 1) * GROUP],
                    ps[:, :, :],
                    axis=mybir.AxisListType.X,
                    op=mybir.AluOpType.max,
                )
            nc.sync.dma_start(cm_out[j], cm_sb[:])

    return nc


def _get_compiled():
    global _compiled
    if _compiled is None:
        _compiled = _build_bass()
    return _compiled


def _run_device(ref_f32: np.ndarray, src_f32: np.ndarray, trace: bool = False):
    """Run the SPMD bass kernel; returns cm [N, N_CHUNKS] fp32 and the raw results obj."""
    from concourse.bass_utils import run_bass_kernel_spmd

    nc = _get_compiled()

    ref_bf = ref_f32.astype(ml_dtypes.bfloat16)
    src_bf = src_f32.astype(ml_dtypes.bfloat16)

    # [C, M] transposed layouts, k-tiled on partitions
    src_tt = np.ascontiguousarray(src_bf.T).reshape(2, 128, M)
    src_tt = np.ascontiguousarray(src_tt.reshape(2, 128, 4, M // 4).transpose(0, 2, 1, 3))

    in_maps = []
    for c in range(N_CORES):
        rows = slice(c * ROWS_PER_CORE, (c + 1) * ROWS_PER_CORE)
        reft = np.ascontiguousarray(ref_bf[rows].T).reshape(2, 128, ROWS_PER_CORE)
        in_maps.append({"ref_t": reft, "src_t": src_tt})

    res = run_bass_kernel_spmd(nc, in_maps, core_ids=list(range(N_CORES)), trace=trace)

    # cm[j, p, i] -> local row = i*128 + p
    cm = np.empty((N, N_CHUNKS), dtype=np.float32)
    for c in range(N_CORES):
        block = res.results[c]["cm"]            # [16, 128, 8]
        cm[c * ROWS_PER_CORE:(c + 1) * ROWS_PER_CORE] = (
            block.transpose(2, 1, 0).reshape(ROWS_PER_CORE, N_CHUNKS)
        )
    return cm, res


def _recompute_chunks(ref64, src64, pairs):
    """Exact fp64 sims for a set of (row, chunk) pairs.

    Returns dict chunk -> (rows_array, values [len(rows), CHUNK])."""
    out = {}
    pairs = np.asarray(pairs)
    if pairs.size == 0:
        return out
    for j in np.unique(pairs[:, 1]):
        rows = pairs[pairs[:, 1] == j, 0]
        vals = ref64[rows] @ src64[j * CHUNK:(j + 1) * CHUNK].T
        out[int(j)] = (rows, vals)
    return out


def kernel(ref_feats, src_feats, num_proposal):
    ref = np.asarray(ref_feats, dtype=np.float32)
    src = np.asarray(src_feats, dtype=np.float32)
    k = int(num_proposal)

    cm, _ = _run_device(ref, src)

    ref64 = ref.astype(np.float64)
    src64 = src.astype(np.float64)

    # ---- per-row argmax over src (all_ref_corr_indices) ----
    row_best = cm.max(axis=1)
    cand_mask = cm >= (row_best[:, None] - MARGIN)
    rows_r, chunks_r = np.nonzero(cand_mask)
    rec = _recompute_chunks(ref64, src64, np.stack([rows_r, chunks_r], axis=1))
    best_val = np.full(N, -np.inf)
    best_idx = np.zeros(N, dtype=np.int64)
    for j, (rows, vals) in sorted(rec.items()):
        am = vals.argmax(axis=1)
        v = vals[np.arange(len(rows)), am]
        idx = j * CHUNK + am
        upd = v > best_val[rows]
        # strict > keeps the lowest column index on exact ties because
        # chunks are visited in ascending order and argmax takes the first max
        best_val[rows] = np.where(upd, v, best_val[rows])
        best_idx[rows] = np.where(upd, idx, best_idx[rows])
    all_ref_corr_indices = best_idx.astype(np.int32)

    # ---- global top-k ----
    flat_cm = cm.ravel()
    kth = min(k, flat_cm.size)
    t = np.partition(flat_cm, flat_cm.size - kth)[flat_cm.size - kth]
    rows_g, chunks_g = np.nonzero(cm >= t - MARGIN)
    rec = _recompute_chunks(ref64, src64, np.stack([rows_g, chunks_g], axis=1))
    cand_vals = []
    cand_flat = []
    for j, (rows, vals) in sorted(rec.items()):
        cols = j * CHUNK + np.arange(CHUNK)
        cand_vals.append(vals.ravel())
        cand_flat.append((rows[:, None] * M + cols[None, :]).ravel())
    cand_vals = np.concatenate(cand_vals)
    cand_flat = np.concatenate(cand_flat)

    # top-k by value desc, ties -> lower flat index (matches jax.lax.top_k)
    order = np.lexsort((cand_flat, -cand_vals))[:k]
    top_flat = cand_flat[order]
    top_vals = cand_vals[order]

    ref_corr_indices = (top_flat // M).astype(np.int32)
    src_corr_indices = (top_flat % M).astype(np.int32)
    corr_scores = np.exp(2.0 * top_vals - 2.0).astype(np.float32)

    return ref_corr_indices, src_corr_indices, corr_scores, all_ref_corr_indices


# revision 4
# speedup vs baseline: 1.2047x; 1.2047x over previous
"""CoarseMatching (retrieval kNN) kernel for 8x Trainium2 NeuronCores.

Problem: ref[8192,256], src[8192,256] (unit-norm rows, fp32).
  sim = ref @ src.T                      [8192, 8192]
  scores = exp(2*sim - 2)                (monotone in sim)
  outputs: global top-k (k=num_proposal) of scores (row idx, col idx, score)
           + per-row argmax over src.

Strategy:
  - Shard ref rows across 8 cores (1024 rows each); src replicated.
  - Device (per core): bf16 matmul (fp32 PSUM accumulation) of its
    [1024 x 8192] sim block; DVE max-reduce each PSUM group to per-row,
    per-512-column-chunk maxes "cm" [1024 x 16]. Only cm leaves the device.
  - Host: candidate selection from cm with a safety margin that dominates
    the bf16 rounding error, then exact fp64 recomputation of only the
    few hundred candidate chunks (BLAS) for exact top-k / argmax.

  Device cm error vs true fp32 sims is bounded by bf16 input rounding
  (~6e-4 absolute); MARGIN=2e-2 makes candidate selection exact.
"""

import sys

sys.path.insert(0, "/opt/trn_rl_repo")

import numpy as np
import ml_dtypes

N_CORES = 8
N, M, C = 8192, 8192, 256
ROWS_PER_CORE = N // N_CORES          # 1024
STRIPS = ROWS_PER_CORE // 128         # 8 strips of 128 rows
CHUNK = 512                           # column chunk = one PSUM bank of fp32
N_CHUNKS = M // CHUNK                 # 16
GROUP = 4                             # PSUM banks per reduce group
MARGIN = 2e-2                         # >> bf16 matmul error (~6e-4)

_compiled = None


def _build_bass():
    from contextlib import ExitStack
    import concourse.bass as bass
    import concourse.bacc as bacc
    import concourse.tile as tile
    from concourse import mybir

    nc = bacc.Bacc("TRN2", target_bir_lowering=False, debug=False)
    bf16 = mybir.dt.bfloat16
    f32 = mybir.dt.float32

    # lhsT k-tiles: [2, 128, 1024] (contract dim on partitions)
    ref_t = nc.declare_dram_parameter("ref_t", [2, 128, ROWS_PER_CORE], bf16, isOutput=False)
    # rhs k-tiles quartered for load/compute overlap: [2, 4, 128, 2048]
    src_t = nc.declare_dram_parameter("src_t", [2, 4, 128, M // 4], bf16, isOutput=False)
    # out: per chunk j, [128 partitions, 8 strips] of chunk maxes
    cm_out = nc.declare_dram_parameter("cm", [N_CHUNKS, 128, STRIPS], f32, isOutput=True)

    with tile.TileContext(nc) as tc, ExitStack() as ctx:
        sbuf = ctx.enter_context(tc.tile_pool(name="sbuf", bufs=1))
        cm_pool = ctx.enter_context(tc.tile_pool(name="cmp", bufs=4))
        psum = ctx.enter_context(tc.tile_pool(name="psum", bufs=2, space="PSUM"))

        # resident weights (ref^T) per k-tile
        reft = [sbuf.tile([128, ROWS_PER_CORE], bf16, name=f"reft{k}") for k in range(2)]
        for k in range(2):
            nc.sync.dma_start(reft[k][:], ref_t[k])

        # resident src^T quarters per k-tile
        srcq = [
            [sbuf.tile([128, M // 4], bf16, name=f"srcq{k}_{q}") for q in range(4)]
            for k in range(2)
        ]
        for q in range(4):
            for k in range(2):
                nc.sync.dma_start(srcq[k][q][:], src_t[k, q])

        for j in range(N_CHUNKS):          # column chunks of 512
            q, off = j // 4, (j % 4) * CHUNK
            cm_sb = cm_pool.tile([128, STRIPS], f32, name="cm_sb", tag="cm_sb")
            for h in range(STRIPS // GROUP):   # two strip-halves
                ps = psum.tile([128, GROUP, CHUNK], f32, name="ps", tag="ps")
                for s in range(GROUP):
                    strip = h * GROUP + s
                    for k in range(2):
                        nc.tensor.matmul(
                            ps[:, s],
                            reft[k][:, strip * 128:(strip + 1) * 128],
                            srcq[k][q][:, off:off + CHUNK],
                            start=(k == 0),
                            stop=(k == 1),
                        )
                nc.vector.tensor_reduce(
                    cm_sb[:, h * GROUP:(h + 1) * GROUP],
                    ps[:, :, :],
                    axis=mybir.AxisListType.X,
                    op=mybir.AluOpType.max,
                )
            nc.sync.dma_start(cm_out[j], cm_sb[:])

    nc.compile()
    return nc


def _get_compiled():
    global _compiled
    if _compiled is None:
        _compiled = _build_bass()
    return _compiled


def _ensure_ntff_hook():
    """Register the axon NTFF profiling hook if the image's antenv lacks it."""
    try:
        from antenv.axon_hooks import get_axon_ntff_profile_hook  # noqa: F401
        return
    except ImportError:
        pass
    try:
        import types

        sys.path.insert(0, "/root/.axon_site")
        from trn_agent_boot.trn_boot import _ntff_profile_via_ctypes

        hook = _ntff_profile_via_ctypes("/opt/axon/libaxon_pjrt.so")
        m = types.ModuleType("antenv.axon_hooks")
        m._hook = hook
        m.get_axon_ntff_profile_hook = lambda: m._hook
        m.set_axon_ntff_profile_hook = lambda h: setattr(m, "_hook", h)
        sys.modules["antenv.axon_hooks"] = m
        import antenv

        antenv.axon_hooks = m
    except Exception as e:  # profiling is optional; never break the run
        print(f"NTFF hook registration failed: {e}", file=sys.stderr)


def _run_device(ref_f32: np.ndarray, src_f32: np.ndarray, trace: bool = False):
    """Run the SPMD bass kernel; returns cm [N, N_CHUNKS] fp32 and the raw results obj."""
    from concourse.bass_utils import run_bass_kernel_spmd

    if trace:
        _ensure_ntff_hook()

    nc = _get_compiled()

    ref_bf = ref_f32.astype(ml_dtypes.bfloat16)
    src_bf = src_f32.astype(ml_dtypes.bfloat16)

    # [C, M] transposed layouts, k-tiled on partitions
    src_tt = np.ascontiguousarray(src_bf.T).reshape(2, 128, M)
    src_tt = np.ascontiguousarray(src_tt.reshape(2, 128, 4, M // 4).transpose(0, 2, 1, 3))

    in_maps = []
    for c in range(N_CORES):
        rows = slice(c * ROWS_PER_CORE, (c + 1) * ROWS_PER_CORE)
        reft = np.ascontiguousarray(ref_bf[rows].T).reshape(2, 128, ROWS_PER_CORE)
        in_maps.append({"ref_t": reft, "src_t": src_tt})

    res = run_bass_kernel_spmd(nc, in_maps, core_ids=list(range(N_CORES)), trace=trace)

    # cm[j, p, i] -> local row = i*128 + p
    cm = np.empty((N, N_CHUNKS), dtype=np.float32)
    for c in range(N_CORES):
        block = res.results[c]["cm"]            # [16, 128, 8]
        cm[c * ROWS_PER_CORE:(c + 1) * ROWS_PER_CORE] = (
            block.transpose(2, 1, 0).reshape(ROWS_PER_CORE, N_CHUNKS)
        )
    return cm, res


def _recompute_chunks(ref64, src64, pairs):
    """Exact fp64 sims for a set of (row, chunk) pairs.

    Returns dict chunk -> (rows_array, values [len(rows), CHUNK])."""
    out = {}
    pairs = np.asarray(pairs)
    if pairs.size == 0:
        return out
    for j in np.unique(pairs[:, 1]):
        rows = pairs[pairs[:, 1] == j, 0]
        vals = ref64[rows] @ src64[j * CHUNK:(j + 1) * CHUNK].T
        out[int(j)] = (rows, vals)
    return out


def kernel(ref_feats, src_feats, num_proposal):
    ref = np.asarray(ref_feats, dtype=np.float32)
    src = np.asarray(src_feats, dtype=np.float32)
    k = int(num_proposal)

    cm, _ = _run_device(ref, src)

    ref64 = ref.astype(np.float64)
    src64 = src.astype(np.float64)

    # ---- per-row argmax over src (all_ref_corr_indices) ----
    row_best = cm.max(axis=1)
    cand_mask = cm >= (row_best[:, None] - MARGIN)
    rows_r, chunks_r = np.nonzero(cand_mask)
    rec = _recompute_chunks(ref64, src64, np.stack([rows_r, chunks_r], axis=1))
    best_val = np.full(N, -np.inf)
    best_idx = np.zeros(N, dtype=np.int64)
    for j, (rows, vals) in sorted(rec.items()):
        am = vals.argmax(axis=1)
        v = vals[np.arange(len(rows)), am]
        idx = j * CHUNK + am
        upd = v > best_val[rows]
        # strict > keeps the lowest column index on exact ties because
        # chunks are visited in ascending order and argmax takes the first max
        best_val[rows] = np.where(upd, v, best_val[rows])
        best_idx[rows] = np.where(upd, idx, best_idx[rows])
    all_ref_corr_indices = best_idx.astype(np.int32)

    # ---- global top-k ----
    flat_cm = cm.ravel()
    kth = min(k, flat_cm.size)
    t = np.partition(flat_cm, flat_cm.size - kth)[flat_cm.size - kth]
    rows_g, chunks_g = np.nonzero(cm >= t - MARGIN)
    rec = _recompute_chunks(ref64, src64, np.stack([rows_g, chunks_g], axis=1))
    cand_vals = []
    cand_flat = []
    for j, (rows, vals) in sorted(rec.items()):
        cols = j * CHUNK + np.arange(CHUNK)
        cand_vals.append(vals.ravel())
        cand_flat.append((rows[:, None] * M + cols[None, :]).ravel())
    cand_vals = np.concatenate(cand_vals)
    cand_flat = np.concatenate(cand_flat)

    # top-k by value desc, ties -> lower flat index (matches jax.lax.top_k)
    order = np.lexsort((cand_flat, -cand_vals))[:k]
    top_flat = cand_flat[order]
    top_vals = cand_vals[order]

    ref_corr_indices = (top_flat // M).astype(np.int32)
    src_corr_indices = (top_flat % M).astype(np.int32)
    corr_scores = np.exp(2.0 * top_vals - 2.0).astype(np.float32)

    return ref_corr_indices, src_corr_indices, corr_scores, all_ref_corr_indices
